# revision 1
# baseline (speedup 1.0000x reference)
"""CategoryConsistencyLoss kernel for 8 trn2 NeuronCores.

loss = mean_i clip(||x_i - w_{labels_i}||^2, 1e-12, 1e12)

The reference materializes the full [N, C] squared-distance matrix and then
gathers the label-indexed diagonal entries; only those N entries matter, so
the kernel gathers w_{labels_i} rows with indirect DMA and computes the
row-wise squared distance directly (O(N*D) instead of O(N*C*D)).

Sharding: data-parallel over N across the 8 cores; weightcenters replicated.
Each core returns per-row distances; the host does the final clip + mean.
"""

import numpy as np

import concourse.bacc as bacc
import concourse.bass as bass
import concourse.mybir as mybir
import concourse.tile as tile
from concourse import bass_utils

N, C, D = 16384, 1000, 2048
N_CORES = 8
N_LOC = N // N_CORES  # 2048 rows per core
P = 128               # SBUF partitions
T = N_LOC // P        # 16 tiles per core

_nc_cache = None
LAST_RESULTS = None  # BassKernelResults of the most recent run (for profiling)


def _build():
    nc = bacc.Bacc("TRN2", target_bir_lowering=False, debug=False)
    x_d = nc.dram_tensor("x", [N_LOC, D], mybir.dt.float32, kind="ExternalInput")
    lab_d = nc.dram_tensor("labels", [P, T], mybir.dt.int32, kind="ExternalInput")
    w_d = nc.dram_tensor("w", [C, D], mybir.dt.float32, kind="ExternalInput")
    out_d = nc.dram_tensor("dist", [P, T], mybir.dt.float32, kind="ExternalOutput")

    x_ap = x_d.ap()
    w_ap = w_d.ap()

    with tile.TileContext(nc) as tc:
        with (
            tc.tile_pool(name="main", bufs=4) as pool,
            tc.tile_pool(name="small", bufs=1) as spool,
        ):
            lab_sb = spool.tile([P, T], mybir.dt.int32)
            nc.sync.dma_start(out=lab_sb[:], in_=lab_d.ap()[:])
            rowsum = spool.tile([P, T], mybir.dt.float32)
            for t in range(T):
                x_t = pool.tile([P, D], mybir.dt.float32, tag="x")
                wg_t = pool.tile([P, D], mybir.dt.float32, tag="wg")
                nc.sync.dma_start(out=x_t[:], in_=x_ap[t * P : (t + 1) * P, :])
                nc.gpsimd.indirect_dma_start(
                    out=wg_t[:],
                    out_offset=None,
                    in_=w_ap[:],
                    in_offset=bass.IndirectOffsetOnAxis(ap=lab_sb[:, t : t + 1], axis=0),
                )
                nc.vector.tensor_tensor(
                    out=x_t[:], in0=x_t[:], in1=wg_t[:], op=mybir.AluOpType.subtract
                )
                nc.scalar.activation(
                    out=x_t[:],
                    in_=x_t[:],
                    func=mybir.ActivationFunctionType.Square,
                    accum_out=rowsum[:, t : t + 1],
                )
            nc.sync.dma_start(out=out_d.ap()[:], in_=rowsum[:])
    nc.compile()
    return nc


def kernel(x, labels, weightcenters):
    global _nc_cache, LAST_RESULTS
    x = np.ascontiguousarray(np.asarray(x, dtype=np.float32))
    labels = np.asarray(labels, dtype=np.int32)
    w = np.ascontiguousarray(np.asarray(weightcenters, dtype=np.float32))

    if _nc_cache is None:
        _nc_cache = _build()
    nc = _nc_cache

    in_maps = []
    for c in range(N_CORES):
        xs = x[c * N_LOC : (c + 1) * N_LOC]
        # column t of the [P, T] label tile holds labels[t*P:(t+1)*P]
        ls = np.ascontiguousarray(labels[c * N_LOC : (c + 1) * N_LOC].reshape(T, P).T)
        in_maps.append({"x": xs, "labels": ls, "w": w})

    res = bass_utils.run_bass_kernel_spmd(nc, in_maps, core_ids=list(range(N_CORES)))
    LAST_RESULTS = res

    dist = np.concatenate(
        [res.results[c]["dist"].T.reshape(-1) for c in range(N_CORES)]
    )
    loss = np.clip(dist.astype(np.float64), 1e-12, 1e12).sum() / N
    return np.float32(loss)
